# revision 21
# baseline (speedup 1.0000x reference)
"""Trainium2 Bass kernel for nn_CombinedLoss (surface loss + Tversky loss).

The reference computes a 4D (C,D,H,W) Euclidean distance transform of the
one-hot argmax mask per batch element, but because the EDT includes the
channel axis (C=3) the distance maps collapse analytically:

  * pos_d == 1 at every pos voxel (a zero channel-neighbor always exists at
    distance 1), so the (pos_d - 1) * pos term is identically zero.
  * neg_d at channel 1 (the only channel SurfaceLoss reads, idc=[1]) is
    sqrt(min(spatial_dist^2_to_cls1, 1)) == 1 at every voxel with cls != 1.

  => dist_maps[:, 1] == (argmax_c probs != 1), exactly (verified vs scipy EDT).

So the whole loss is elementwise work + global reductions:

  surface = mean(p1 * [argmax != 1])        over B*D*H*W voxels
  tversky = 1 - (tp + 1) / (0.5*(sum(p)+sum(t)) + 1),   tp = sum(p*t)

Inputs are shipped to the device as bf16. [argmax != 1] = 1[max(p0,p2) >= p1]
would pick up a one-sided bias from bf16 ties, so ties count 1/2:
ind = 0.5*(is_ge + is_gt), giving ~3e-6 total relative error (validated on
the exact reference inputs on host).

Work split per core (voxels are flattened and split evenly across 8 cores;
host does the final tiny reduction in f64):
  * DVE: m=max(p0,p2); a=is_ge(m,p1); b=is_gt(m,p1); accumulate p1*a, p1*b
    via scalar_tensor_tensor(bypass,mult,accum_out).
  * ACT: issues the t-plane DMA triggers (both SP and ACT can trigger HWDGE,
    halving DMA trigger serialization).
  * PE:  warms the HAM clock-gate with dummy matmuls during the DMA head
    (cold PE runs at 1.2 GHz; ~4us of sustained activity unlocks 2.4 GHz),
    then tp via the diagonal trick (psa[128,129] += p_tile^T @ [t_tile|ones]
    over all channel/voxel tiles: diag = p*t partials, col 128 = sum(p)),
    and sum(t) via ones-column stationary streams (psb[1,387]; baked ones
    columns are subtracted on host).
Raw Bass with standalone waits (this toolchain rejects instructions carrying
more than one attached sync-wait).
"""

import numpy as np
import ml_dtypes

import concourse.bass as bass
import concourse.mybir as mybir
from concourse.bass_utils import run_bass_kernel_spmd

N_CORES = 8
B, C, D, H, W = 2, 3, 64, 128, 128
N_VOX = B * D * H * W            # 2_097_152
VOX_PER_CORE = N_VOX // N_CORES  # 262_144
P = 128                          # partitions
NCH = 4                          # chunks per core
CW = VOX_PER_CORE // (P * NCH)   # 512 columns per chunk
TPC = CW // P                    # 4 PE tiles per chunk per channel
PW = C * CW                      # 1536 p-columns per chunk
TW = C * (CW + TPC)              # 1548 t-columns per chunk (ones baked in)
N_ONES = C * NCH * TPC * P       # total baked-ones contribution to psb: 6144
N_WARM = 14                      # dummy 512-col matmuls to ramp the PE clock

_CACHE = {}


def _build_module():
    from contextlib import ExitStack

    Alu = mybir.AluOpType
    f32 = mybir.dt.float32
    bf16 = mybir.dt.bfloat16

    nc = bass.Bass()
    p_in = nc.dram_tensor("p", [NCH, P, PW], bf16, kind="ExternalInput")
    t_in = nc.dram_tensor("t", [NCH, P, TW], bf16, kind="ExternalInput")
    s1_out = nc.dram_tensor("s1", [P, NCH * 2], f32, kind="ExternalOutput")
    psa_out = nc.dram_tensor("psa", [P, P + 1], f32, kind="ExternalOutput")
    psb_out = nc.dram_tensor("psb", [1, 387], f32, kind="ExternalOutput")

    with (
        ExitStack() as ctx,
        nc.sbuf_tensor([P, NCH * PW], bf16) as p_sb,
        nc.sbuf_tensor([P, NCH * TW], bf16) as t_sb,
        nc.sbuf_tensor([P, CW], bf16) as warm_sb,
        nc.sbuf_tensor([P, CW], bf16) as m_sb,
        nc.sbuf_tensor([P, CW], bf16) as a_sb,
        nc.sbuf_tensor([P, CW], bf16) as b_sb,
        nc.sbuf_tensor([P, CW], bf16) as vj_a,
        nc.sbuf_tensor([P, CW], bf16) as vj_b,
        nc.sbuf_tensor([P, NCH * 2], f32) as s1_sb,
        nc.sbuf_tensor([P, P + 1], f32) as psa_sb,
        nc.sbuf_tensor([1, 387], f32) as psb_sb,
        nc.psum_tensor([P, P + 1], f32) as psa,
        nc.psum_tensor([1, 387], f32) as psb,
        nc.psum_tensor([P, CW], f32) as psw,
        nc.Block() as block,
    ):
        g_sem = ctx.enter_context(nc.semaphore("g_sem"))
        v_sem = ctx.enter_context(nc.semaphore("v_sem"))
        pe_sem = ctx.enter_context(nc.semaphore("pe_sem"))
        c_sem = ctx.enter_context(nc.semaphore("c_sem"))
        o1_sem = ctx.enter_context(nc.semaphore("o1_sem"))
        o2_sem = ctx.enter_context(nc.semaphore("o2_sem"))
        o3_sem = ctx.enter_context(nc.semaphore("o3_sem"))
        p_sems = [ctx.enter_context(nc.semaphore(f"p_sem{i}")) for i in range(NCH)]
        t_sems = [ctx.enter_context(nc.semaphore(f"t_sem{i}")) for i in range(NCH)]

        def pp(ch, c):
            return p_sb[:, ch * PW + c * CW : ch * PW + (c + 1) * CW]

        def ptile(ch, c, i):
            off = ch * PW + c * CW + i * P
            return p_sb[:, off : off + P]

        def tblock(ch, c, i):
            off = ch * TW + c * (CW + TPC) + i * (P + 1)
            return t_sb[:, off : off + P + 1]

        @block.sync
        def _(sync):
            for ch in range(NCH):
                sync.dma_start(
                    p_sb[:, ch * PW : (ch + 1) * PW], p_in[ch]
                ).then_inc(p_sems[ch], 16)
            sync.wait_ge(v_sem, NCH * 5)
            sync.dma_start(s1_out[:], s1_sb[:]).then_inc(o1_sem, 16)
            sync.wait_ge(c_sem, 2)
            sync.dma_start(psa_out[:], psa_sb[:]).then_inc(o2_sem, 16)
            sync.dma_start(psb_out[:], psb_sb[:]).then_inc(o3_sem, 16)
            sync.wait_ge(o1_sem, 16)
            sync.wait_ge(o2_sem, 16)
            sync.wait_ge(o3_sem, 16)

        @block.scalar
        def _(scalar):
            for ch in range(NCH):
                scalar.dma_start(
                    t_sb[:, ch * TW : (ch + 1) * TW], t_in[ch]
                ).then_inc(t_sems[ch], 16)

        @block.gpsimd
        def _(gpsimd):
            gpsimd.memset(warm_sb[:], 0.0).then_inc(g_sem, 1)

        @block.vector
        def _(vector):
            for ch in range(NCH):
                vector.wait_ge(p_sems[ch], 16)
                if ch:
                    vector.wait_ge(v_sem, 5 * ch)
                vector.tensor_tensor(
                    m_sb[:], pp(ch, 0), pp(ch, 2), Alu.max
                ).then_inc(v_sem, 1)
                vector.wait_ge(v_sem, 5 * ch + 1)
                vector.tensor_tensor(
                    a_sb[:], m_sb[:], pp(ch, 1), Alu.is_ge
                ).then_inc(v_sem, 1)
                vector.tensor_tensor(
                    b_sb[:], m_sb[:], pp(ch, 1), Alu.is_gt
                ).then_inc(v_sem, 1)
                vector.wait_ge(v_sem, 5 * ch + 2)
                vector.scalar_tensor_tensor(
                    vj_a[:], pp(ch, 1), 0.0, a_sb[:], Alu.bypass, Alu.mult,
                    accum_out=s1_sb[:, ch * 2 : ch * 2 + 1],
                ).then_inc(v_sem, 1)
                vector.wait_ge(v_sem, 5 * ch + 3)
                vector.scalar_tensor_tensor(
                    vj_b[:], pp(ch, 1), 0.0, b_sb[:], Alu.bypass, Alu.mult,
                    accum_out=s1_sb[:, ch * 2 + 1 : ch * 2 + 2],
                ).then_inc(v_sem, 1)
            # PSUM -> SBUF copies once PE is done
            vector.wait_ge(pe_sem, 2)
            vector.tensor_copy(psa_sb[:], psa[:]).then_inc(c_sem, 1)
            vector.tensor_copy(psb_sb[:], psb[:]).then_inc(c_sem, 1)

        @block.tensor
        def _(tensor):
            # HAM warmup: dummy matmuls on zeroed scratch while DMAs land.
            tensor.wait_ge(g_sem, 1)
            for w in range(N_WARM):
                nc.tensor.matmul(
                    psw[:], warm_sb[:, :P], warm_sb[:], start=True, stop=True
                )
            n_tp = NCH * C * TPC          # 48 tp matmuls
            n_st = NCH * 4                # 16 sum(t) matmuls (387 cols each)
            k_tp = k_st = 0
            ones_col = t_sb[:, P : P + 1]  # any baked ones column
            for ch in range(NCH):
                tensor.wait_ge(p_sems[ch], 16)
                tensor.wait_ge(t_sems[ch], 16)
                for c in range(C):
                    for i in range(TPC):
                        mm = nc.tensor.matmul(
                            psa[:],
                            ptile(ch, c, i),
                            tblock(ch, c, i),
                            start=(k_tp == 0),
                            stop=(k_tp == n_tp - 1),
                        )
                        if k_tp == n_tp - 1:
                            mm.then_inc(pe_sem, 1)
                        k_tp += 1
                for q in range(4):
                    off = ch * TW + q * 387
                    mm = nc.tensor.matmul(
                        psb[:],
                        ones_col,
                        t_sb[:, off : off + 387],
                        start=(k_st == 0),
                        stop=(k_st == n_st - 1),
                    )
                    if k_st == n_st - 1:
                        mm.then_inc(pe_sem, 1)
                    k_st += 1

    return nc


def _shard(probs, target):
    """f32 [B,C,D,H,W] x2 -> per-core bf16 arrays:
    p [NCH, P, C*CW] and t [NCH, P, C*(CW+TPC)] (ones columns baked in)."""
    pf = np.ascontiguousarray(probs.transpose(1, 0, 2, 3, 4)).reshape(C, N_VOX)
    tf = np.ascontiguousarray(target.transpose(1, 0, 2, 3, 4)).reshape(C, N_VOX)
    out = []
    for k in range(N_CORES):
        sl = slice(k * VOX_PER_CORE, (k + 1) * VOX_PER_CORE)
        pk = pf[:, sl].reshape(C, P, NCH, CW).transpose(2, 1, 0, 3)
        pk = np.ascontiguousarray(pk).astype(ml_dtypes.bfloat16)
        tk4 = tf[:, sl].reshape(C, P, NCH, TPC, P).transpose(2, 1, 0, 3, 4)
        tk = np.ones((NCH, P, C, TPC, P + 1), dtype=ml_dtypes.bfloat16)
        tk[..., :P] = tk4.astype(ml_dtypes.bfloat16)
        out.append(
            (
                pk.reshape(NCH, P, PW),
                np.ascontiguousarray(tk.reshape(NCH, P, TW)),
            )
        )
    return out


def _finalize(results):
    s1 = tp = sp = st = 0.0
    for r in results:
        s1 += r["s1"].astype(np.float64).sum()
        psa = r["psa"].astype(np.float64)
        tp += np.diag(psa[:, :P]).sum()
        sp += psa[:, P].sum()
        st += r["psb"].astype(np.float64).sum() - N_ONES
    surface = 0.5 * s1 / float(N_VOX)
    tversky = 1.0 - (tp + 1.0) / (0.5 * (sp + st) + 1.0)
    return np.float32(surface + tversky)


def kernel(probs: np.ndarray, target: np.ndarray) -> np.ndarray:
    probs = np.asarray(probs, dtype=np.float32)
    target = np.asarray(target, dtype=np.float32)

    if "nc" not in _CACHE:
        _CACHE["nc"] = _build_module()
    nc = _CACHE["nc"]

    shards = _shard(probs, target)
    in_maps = [{"p": p, "t": t} for p, t in shards]
    res = run_bass_kernel_spmd(nc, in_maps, core_ids=list(range(N_CORES)))
    return _finalize(res.results)


# revision 25
# speedup vs baseline: 1.0056x; 1.0056x over previous
"""Trainium2 Bass kernel for nn_CombinedLoss (surface loss + Tversky loss).

The reference computes a 4D (C,D,H,W) Euclidean distance transform of the
one-hot argmax mask per batch element, but because the EDT includes the
channel axis (C=3) the distance maps collapse analytically:

  * pos_d == 1 at every pos voxel (a zero channel-neighbor always exists at
    distance 1), so the (pos_d - 1) * pos term is identically zero.
  * neg_d at channel 1 (the only channel SurfaceLoss reads, idc=[1]) is
    sqrt(min(spatial_dist^2_to_cls1, 1)) == 1 at every voxel with cls != 1.

  => dist_maps[:, 1] == (argmax_c probs != 1), exactly (verified vs scipy EDT).

So the whole loss is elementwise work + global reductions:

  surface = mean(p1 * [argmax != 1])        over B*D*H*W voxels
  tversky = 1 - (tp + 1) / (0.5*(sum(p)+sum(t)) + 1),   tp = sum(p*t)

Inputs are shipped to the device as bf16. [argmax != 1] = 1[max(p0,p2) >= p1]
would pick up a one-sided bias from bf16 ties, so ties count 1/2:
ind = 0.5*(is_ge + is_gt), giving ~3e-6 total relative error (validated on
the exact reference inputs on host).

Per core (voxels are flattened and split evenly across the 8 cores):
  * DMA: each chunk's p/t arrives as two partition-half DMAs spread over the
    three available DMA issue paths (SP-HWDGE, ACT-HWDGE, Pool-SWDGE) --
    single-queue streaming measured ~120 GB/s, so three queues are needed to
    approach the ~360 GB/s core budget.
  * DVE: m=max(p0,p2); a=is_ge(m,p1); b=is_gt(m,p1); accumulate p1*a, p1*b
    via scalar_tensor_tensor(bypass,mult,accum_out).
  * PE:  warms the HAM clock-gate with dummy matmuls during the DMA head,
    then tp via the diagonal trick (psa[128,129] += p_tile^T @ [t_tile|ones],
    diag = p*t partials, col 128 = sum(p)) and sum(t) via ones-stationary
    streams (psb[1,387]; baked ones columns subtracted on host).
  * Final: diag extracted on DVE with a baked identity tile, then one fp32
    ones^T matmul collapses all per-partition stats to a single row so the
    output DMA is one descriptor. Host does the last ~400-element reduce.
Raw Bass with standalone waits (this toolchain rejects instructions carrying
more than one attached sync-wait).
"""

import numpy as np
import ml_dtypes

import concourse.bass as bass
import concourse.mybir as mybir
from concourse.bass_utils import run_bass_kernel_spmd

N_CORES = 8
B, C, D, H, W = 2, 3, 64, 128, 128
N_VOX = B * D * H * W            # 2_097_152
VOX_PER_CORE = N_VOX // N_CORES  # 262_144
P = 128                          # partitions
NCH = 4                          # chunks per core
CW = VOX_PER_CORE // (P * NCH)   # 512 columns per chunk
TPC = CW // P                    # 4 PE tiles per chunk per channel
PW = C * CW + P                  # 1664 p-cols per chunk (identity/pad baked)
TW = C * (CW + TPC)              # 1548 t-columns per chunk (ones baked in)
N_ONES = C * NCH * TPC * P       # total baked-ones contribution to psb: 6144
N_WARM = 8                       # dummy 512-col matmuls to ramp the PE clock
FIN = 10 + 387                   # packed output row

_CACHE = {}


def _build_module():
    from contextlib import ExitStack

    Alu = mybir.AluOpType
    Act = mybir.ActivationFunctionType
    f32 = mybir.dt.float32
    bf16 = mybir.dt.bfloat16

    nc = bass.Bass()
    p_in = nc.dram_tensor("p", [NCH, P, PW], bf16, kind="ExternalInput")
    t_in = nc.dram_tensor("t", [NCH, P, TW], bf16, kind="ExternalInput")
    fin_out = nc.dram_tensor("fin", [1, FIN], f32, kind="ExternalOutput")

    with (
        ExitStack() as ctx,
        nc.sbuf_tensor([P, NCH * PW], bf16) as p_sb,
        nc.sbuf_tensor([P, NCH * TW], bf16) as t_sb,
        nc.sbuf_tensor([P, CW], bf16) as warm_sb,
        nc.sbuf_tensor([P, 1], f32) as ones32,
        nc.sbuf_tensor([P, CW], bf16) as m_sb,
        nc.sbuf_tensor([P, CW], bf16) as a_sb,
        nc.sbuf_tensor([P, CW], bf16) as b_sb,
        nc.sbuf_tensor([P, CW], bf16) as vj_a,
        nc.sbuf_tensor([P, CW], bf16) as vj_b,
        nc.sbuf_tensor([P, 10], f32) as s1x_sb,
        nc.sbuf_tensor([P, P + 1], f32) as psa_sb,
        nc.sbuf_tensor([1, FIN], f32) as fin_sb,
        nc.psum_tensor([P, P + 1], f32) as psa,
        nc.psum_tensor([1, 387], f32) as psb,
        nc.psum_tensor([P, CW], f32) as psw,
        nc.psum_tensor([1, 10], f32) as psf,
        nc.Block(no_gpsimd_drain=True) as block,
    ):
        g_sem = ctx.enter_context(nc.semaphore("g_sem"))
        v_sem = ctx.enter_context(nc.semaphore("v_sem"))
        pe_sem = ctx.enter_context(nc.semaphore("pe_sem"))
        c_sem = ctx.enter_context(nc.semaphore("c_sem"))
        o1_sem = ctx.enter_context(nc.semaphore("o1_sem"))
        # one sem per half-DMA: a sem may not mix SWDGE and HWDGE updaters
        p_sems = [
            [ctx.enter_context(nc.semaphore(f"p_sem{i}_{h}")) for h in range(2)]
            for i in range(NCH)
        ]
        t_sems = [
            [ctx.enter_context(nc.semaphore(f"t_sem{i}_{h}")) for h in range(2)]
            for i in range(NCH)
        ]

        def pp(ch, c):
            return p_sb[:, ch * PW + c * CW : ch * PW + (c + 1) * CW]

        def ptile(ch, c, i):
            off = ch * PW + c * CW + i * P
            return p_sb[:, off : off + P]

        def tblock(ch, c, i):
            off = ch * TW + c * (CW + TPC) + i * (P + 1)
            return t_sb[:, off : off + P + 1]

        identity = p_sb[:, C * CW : C * CW + P]  # chunk 0's extra cols

        # 16 half-DMAs (p/t x chunk x partition-half) spread over the three
        # DMA issue paths; each chunk sem gets 2 x 16 -> consumers wait 32.
        jobs = {"sync": [], "scalar": [], "gpsimd": []}
        order = ["sync", "scalar", "gpsimd"]
        k = 0
        for ch in range(NCH):
            for which in ("p", "t"):
                for half in range(2):
                    jobs[order[k % 3]].append((which, ch, half))
                    k += 1

        def issue(engine, joblist):
            for which, ch, half in joblist:
                rows = slice(half * (P // 2), (half + 1) * (P // 2))
                if which == "p":
                    engine.dma_start(
                        p_sb[rows, ch * PW : (ch + 1) * PW], p_in[ch, rows]
                    ).then_inc(p_sems[ch][half], 16)
                else:
                    engine.dma_start(
                        t_sb[rows, ch * TW : (ch + 1) * TW], t_in[ch, rows]
                    ).then_inc(t_sems[ch][half], 16)

        @block.sync
        def _(sync):
            issue(sync, jobs["sync"])
            sync.wait_ge(c_sem, 2)
            sync.dma_start(fin_out[:], fin_sb[:]).then_inc(o1_sem, 16)
            sync.wait_ge(o1_sem, 16)

        @block.scalar
        def _(scalar):
            issue(scalar, jobs["scalar"])
            # ones32 = 1.0 (scale=0 -> input not read); zero the warmup tile
            scalar.activation(ones32[:], ones32[:], Act.Copy, bias=1.0, scale=0.0)
            scalar.memzero(warm_sb[:]).then_inc(g_sem, 1)

        @block.gpsimd
        def _(gpsimd):
            issue(gpsimd, jobs["gpsimd"])

        @block.vector
        def _(vector):
            for ch in range(NCH):
                vector.wait_ge(p_sems[ch][0], 16)
                vector.wait_ge(p_sems[ch][1], 16)
                if ch:
                    vector.wait_ge(v_sem, 5 * ch)
                vector.tensor_tensor(
                    m_sb[:], pp(ch, 0), pp(ch, 2), Alu.max
                ).then_inc(v_sem, 1)
                vector.wait_ge(v_sem, 5 * ch + 1)
                vector.tensor_tensor(
                    a_sb[:], m_sb[:], pp(ch, 1), Alu.is_ge
                ).then_inc(v_sem, 1)
                vector.tensor_tensor(
                    b_sb[:], m_sb[:], pp(ch, 1), Alu.is_gt
                ).then_inc(v_sem, 1)
                vector.wait_ge(v_sem, 5 * ch + 2)
                vector.scalar_tensor_tensor(
                    vj_a[:], pp(ch, 1), 0.0, a_sb[:], Alu.bypass, Alu.mult,
                    accum_out=s1x_sb[:, ch * 2 : ch * 2 + 1],
                ).then_inc(v_sem, 1)
                vector.wait_ge(v_sem, 5 * ch + 3)
                vector.scalar_tensor_tensor(
                    vj_b[:], pp(ch, 1), 0.0, b_sb[:], Alu.bypass, Alu.mult,
                    accum_out=s1x_sb[:, ch * 2 + 1 : ch * 2 + 2],
                ).then_inc(v_sem, 1)
            # stats assembly: psa -> SBUF, diag via identity, sum(p) column
            vector.wait_ge(v_sem, 5 * NCH)  # own-engine WAW ordering for vj_a
            vector.wait_ge(pe_sem, 2)
            vector.tensor_copy(psa_sb[:], psa[:]).then_inc(v_sem, 1)
            vector.wait_ge(v_sem, 5 * NCH + 1)
            vector.scalar_tensor_tensor(
                vj_a[:, :P], psa_sb[:, :P], 0.0, identity, Alu.bypass, Alu.mult,
                accum_out=s1x_sb[:, 8:9],
            ).then_inc(v_sem, 1)
            vector.tensor_copy(s1x_sb[:, 9:10], psa_sb[:, P : P + 1]).then_inc(
                v_sem, 1
            )
            # psb row can be copied straight to the packed output row
            vector.tensor_copy(fin_sb[:, 10:], psb[:]).then_inc(c_sem, 1)
            # final collapse result
            vector.wait_ge(pe_sem, 3)
            vector.tensor_copy(fin_sb[:, :10], psf[:]).then_inc(c_sem, 1)

        @block.tensor
        def _(tensor):
            # HAM warmup on zeroed scratch while the DMAs land.
            tensor.wait_ge(g_sem, 1)
            for _ in range(N_WARM):
                nc.tensor.matmul(
                    psw[:], warm_sb[:, :P], warm_sb[:], start=True, stop=True
                )
            n_tp = NCH * C * TPC          # 48 tp matmuls
            n_st = NCH * 4                # 16 sum(t) matmuls (387 cols each)
            k_tp = k_st = 0
            ones_col = t_sb[:, P : P + 1]  # any baked ones column
            for ch in range(NCH):
                tensor.wait_ge(p_sems[ch][0], 16)
                tensor.wait_ge(p_sems[ch][1], 16)
                tensor.wait_ge(t_sems[ch][0], 16)
                tensor.wait_ge(t_sems[ch][1], 16)
                for c in range(C):
                    for i in range(TPC):
                        mm = nc.tensor.matmul(
                            psa[:],
                            ptile(ch, c, i),
                            tblock(ch, c, i),
                            start=(k_tp == 0),
                            stop=(k_tp == n_tp - 1),
                        )
                        if k_tp == n_tp - 1:
                            mm.then_inc(pe_sem, 1)
                        k_tp += 1
                for q in range(4):
                    off = ch * TW + q * 387
                    mm = nc.tensor.matmul(
                        psb[:],
                        ones_col,
                        t_sb[:, off : off + 387],
                        start=(k_st == 0),
                        stop=(k_st == n_st - 1),
                    )
                    if k_st == n_st - 1:
                        mm.then_inc(pe_sem, 1)
                    k_st += 1
            # collapse all per-partition stats to one row (fp32 matmul)
            tensor.wait_ge(v_sem, 5 * NCH + 3)
            nc.tensor.matmul(
                psf[:], ones32[:], s1x_sb[:], start=True, stop=True
            ).then_inc(pe_sem, 1)

    return nc


def _shard(probs, target):
    """f32 [B,C,D,H,W] x2 -> per-core bf16 arrays:
    p [NCH, P, PW] (identity baked into chunk 0's extra cols) and
    t [NCH, P, TW] (ones columns baked in)."""
    pf = np.ascontiguousarray(probs.transpose(1, 0, 2, 3, 4)).reshape(C, N_VOX)
    tf = np.ascontiguousarray(target.transpose(1, 0, 2, 3, 4)).reshape(C, N_VOX)
    eye = np.eye(P, dtype=ml_dtypes.bfloat16)
    out = []
    for k in range(N_CORES):
        sl = slice(k * VOX_PER_CORE, (k + 1) * VOX_PER_CORE)
        pk4 = pf[:, sl].reshape(C, P, NCH, CW).transpose(2, 1, 0, 3)
        pk = np.zeros((NCH, P, PW), dtype=ml_dtypes.bfloat16)
        pk[..., : C * CW] = pk4.reshape(NCH, P, C * CW).astype(ml_dtypes.bfloat16)
        pk[0, :, C * CW :] = eye
        tk4 = tf[:, sl].reshape(C, P, NCH, TPC, P).transpose(2, 1, 0, 3, 4)
        tk = np.ones((NCH, P, C, TPC, P + 1), dtype=ml_dtypes.bfloat16)
        tk[..., :P] = tk4.astype(ml_dtypes.bfloat16)
        out.append((pk, np.ascontiguousarray(tk.reshape(NCH, P, TW))))
    return out


def _finalize(results):
    s1 = tp = sp = st = 0.0
    for r in results:
        f = r["fin"].astype(np.float64)[0]
        s1 += f[0:8].sum()
        tp += f[8]
        sp += f[9]
        st += f[10:].sum() - N_ONES
    surface = 0.5 * s1 / float(N_VOX)
    tversky = 1.0 - (tp + 1.0) / (0.5 * (sp + st) + 1.0)
    return np.float32(surface + tversky)


def kernel(probs: np.ndarray, target: np.ndarray) -> np.ndarray:
    probs = np.asarray(probs, dtype=np.float32)
    target = np.asarray(target, dtype=np.float32)

    if "nc" not in _CACHE:
        _CACHE["nc"] = _build_module()
    nc = _CACHE["nc"]

    shards = _shard(probs, target)
    in_maps = [{"p": p, "t": t} for p, t in shards]
    res = run_bass_kernel_spmd(nc, in_maps, core_ids=list(range(N_CORES)))
    return _finalize(res.results)
